# revision 22
# baseline (speedup 1.0000x reference)
"""Trainium2 Bass kernel for a pre-LN transformer block (B=4, T=2048, E=1024, H=16).

Sharding: 8 cores = 4 batches x 2 token-halves. Each core receives its batch's
full 2048 tokens (rolled so its own 1024 query tokens come first), computes
K/V for all 2048 tokens (redundantly with its pair core) and everything else
(Q, attention, proj, FFN) only for its own 1024 tokens. Zero cross-core
communication; host reassembles the output.

LayerNorm gains/biases are folded into the matmul weights host-side:
  q = LN1n(x) @ (diag(g1) Wq) + (b1_ln @ Wq)   with LN1n(x) = (x-mu)*rstd
so the device only computes (x-mu)*rstd. All matmuls run in bf16 (fp32
accumulation in PSUM); residuals/softmax stay fp32.
"""

import numpy as np
import ml_dtypes

BF = ml_dtypes.bfloat16
F8 = ml_dtypes.float8_e4m3

B, T, E, H, HS, FF = 4, 2048, 1024, 16, 64, 4096
TQ = T // 2          # own query tokens per core
NCORES = 8
EPS = 1e-5
NMT = T // 128       # 16 token tiles (full batch)
NMQ = TQ // 128      # 8 token tiles (own half)
NJE = E // 128       # 8 feature tiles of E
NJF = FF // 128      # 32 feature tiles of FF

_CACHE = {}
TRACE = False        # set by test harness to capture an NTFF profile
LAST_RESULTS = None  # BassKernelResults from the most recent run


def _build():
    import concourse.bacc as bacc
    import concourse.tile as tile
    from concourse import mybir
    from contextlib import ExitStack

    f32 = mybir.dt.float32
    bf16 = mybir.dt.bfloat16
    f8 = mybir.dt.float8e4
    i32 = mybir.dt.int32
    DR = mybir.MatmulPerfMode.DoubleRow
    # Schraudolph-style exp in fp32 bit space: bits = x*2^23/ln2 + (127*2^23-C)
    EXP_C1 = float(2.0 ** 23 / np.log(2.0)) * float(HS) ** -0.5
    EXP_C2 = 127.0 * 2.0 ** 23 - 486411.0
    DVE_EXP_ST = {2, 5, 8, 11, 14}
    AF = mybir.ActivationFunctionType
    OP = mybir.AluOpType

    nc = bacc.Bacc("TRN2", target_bir_lowering=False, debug=False,
                   num_devices=NCORES)

    # ---- DRAM I/O ----
    x_d = nc.declare_dram_parameter("x", [T, E], bf16, isOutput=False)
    wq_d = nc.declare_dram_parameter("wq", [NJE, 128, E], f8, isOutput=False)
    wk_d = nc.declare_dram_parameter("wk", [NJE, 128, E], f8, isOutput=False)
    wv_d = nc.declare_dram_parameter("wv", [NJE, 128, E], f8, isOutput=False)
    wo_d = nc.declare_dram_parameter("wo", [NJE, 128, E], f8, isOutput=False)
    w1_d = nc.declare_dram_parameter("w1", [NJF, 128, E], bf16, isOutput=False)
    w2_d = nc.declare_dram_parameter("w2", [NJF, 128, E], bf16, isOutput=False)
    cq_d = nc.declare_dram_parameter("cq", [128, NJE], f32, isOutput=False)
    ck_d = nc.declare_dram_parameter("ck", [128, NJE], f32, isOutput=False)
    cvb_d = nc.declare_dram_parameter("cvb", [128, E], f32, isOutput=False)
    xq_d = nc.declare_dram_parameter("xq", [TQ, E], f32, isOutput=False)
    b2b_d = nc.declare_dram_parameter("b2b", [128, E], f32, isOutput=False)
    b1c_d = nc.declare_dram_parameter("b1c", [128, NJF], f32, isOutput=False)
    out_d = nc.declare_dram_parameter("out", [TQ, E], f32, isOutput=True)
    rbounce = nc.dram_tensor("rbounce", [H, TQ], f32)

    def layernorm(stats_pool, x_sb, out_bf, eps_sb):
        st = stats_pool.tile([128, 2, 6], f32, name="ln_st")
        nc.vector.bn_stats(out=st[:, 0, :], in_=x_sb[:, 0:512])
        nc.vector.bn_stats(out=st[:, 1, :], in_=x_sb[:, 512:1024])
        mv = stats_pool.tile([128, 2], f32, name="ln_mv")
        nc.vector.bn_aggr(out=mv[:], in_=st[:])
        rstd = stats_pool.tile([128, 1], f32, name="ln_rstd")
        nc.scalar.activation(out=rstd[:], in_=mv[:, 1:2], func=AF.Sqrt,
                             bias=eps_sb[:])
        nc.vector.reciprocal(out=rstd[:], in_=rstd[:])
        # out = x*rstd - mu*rstd on ACT (keeps the big elementwise op off DVE)
        nmr = stats_pool.tile([128, 1], f32, name="ln_nmr")
        nc.vector.tensor_tensor(out=nmr[:], in0=mv[:, 0:1], in1=rstd[:],
                                op=OP.mult)
        nc.vector.tensor_scalar_mul(out=nmr[:], in0=nmr[:], scalar1=-1.0)
        nc.scalar.activation(out=out_bf[:], in_=x_sb[:], func=AF.Identity,
                             bias=nmr[:], scale=rstd[:])

    with tile.TileContext(nc) as tc:
        top = ExitStack()
        const = top.enter_context(tc.tile_pool(name="const", bufs=1, side="left"))
        eps_sb = const.tile([128, 1], f32)
        nc.vector.memset(eps_sb[:], EPS)
        cq_sb = const.tile([128, NJE], f32)
        nc.sync.dma_start(out=cq_sb[:], in_=cq_d[:])
        ck_sb = const.tile([128, NJE], f32)
        nc.sync.dma_start(out=ck_sb[:], in_=ck_d[:])
        cv_sb = const.tile([128, E], f32)
        nc.sync.dma_start(out=cv_sb[:], in_=cvb_d[:])

        # Persistent activations use per-tensor pools whose lifetimes are
        # managed manually (ExitStack.close as soon as the last consumer
        # phase is emitted) so SBUF space is recycled across phases. Pool
        # allocation is a two-sided LIFO stack; sides are chosen so pool
        # lifetimes nest properly.

        # ---------- Phase 1: LN1 + transpose ----------
        hT_es = ExitStack()
        hT_pool = hT_es.enter_context(tc.tile_pool(name="hT", bufs=1, side="left"))
        # 4 tiles of 4 token-tiles each so QKV matmuls can start as soon as
        # the first group of LN1 transposes lands
        hG = [hT_pool.tile([128, 4, NJE, 128], bf16, name=f"hG{g}")
              for g in range(4)]
        h8 = [hT_pool.tile([128, NJE, 4, 128], f8, name=f"h8{g}")
              for g in range(4)]
        with tc.tile_pool(name="ln1", bufs=16, side="left") as xin, \
             tc.tile_pool(name="ln1s", bufs=10, side="left") as stp, \
             tc.tile_pool(name="ln1h", bufs=5, side="left") as hbp:
            xt = []
            for mt in range(NMT):
                x_sb = xin.tile([128, E], bf16)
                nc.sync.dma_start(out=x_sb[:], in_=x_d[mt * 128:(mt + 1) * 128, :])
                xt.append(x_sb)
            for mt in range(NMT):
                h_bf = hbp.tile([128, E], bf16)
                layernorm(stp, xt[mt], h_bf, eps_sb)
                nc.sync.dma_start_transpose(out=hG[mt // 4][:, mt % 4, :, :],
                                            in_=h_bf[:])
                nc.vector.tensor_copy(out=h8[mt // 4][:, :, mt % 4, :],
                                      in_=hG[mt // 4][:, mt % 4, :, :])

        # ---------- Phase 2: QKV projections ----------
        qkv_es = ExitStack()
        qkv_pool = qkv_es.enter_context(tc.tile_pool(name="qkvact", bufs=1, side="right"))
        # q^T is stored zero-padded per head: head h occupies partition rows
        # (h%2)*64..+64 of slice [:, h, :], the other 64 rows are zero. The
        # scores matmul can then contract over the full K=128 partitions
        # (the paired head's k rows hit zeros) which keeps the PE HAM
        # activity monitor at full clock; K=64 matmuls run the array at
        # half activity and HAM throttles the PE to 1.2 GHz.
        qT = qkv_pool.tile([128, H, TQ], bf16)         # q^T (own tokens)
        kT = qkv_pool.tile([128, NJE, T], bf16)        # k^T (all tokens)
        v_aug = qkv_pool.tile([128, NMT, H, HS + 1], bf16)  # v + ones col
        with tc.tile_pool(name="qkv_ps", bufs=8, space="PSUM") as qkps, \
             tc.tile_pool(name="w_pool", bufs=2, side="right") as wqkv:
                wq_sb = wqkv.tile([128, NJE, E], f8, name="wt")
                wk_sb = wqkv.tile([128, NJE, E], f8, name="wt")
                for j in range(NJE):
                    nc.gpsimd.dma_start(out=wq_sb[:, j, :], in_=wq_d[j])
                for j in range(NJE):
                    nc.gpsimd.dma_start(out=wk_sb[:, j, :], in_=wk_d[j])
                for g in range(2):
                    for mf in range(NJE):
                        pq = qkps.tile([128, 512], f32, name="ps_qkv")
                        for j in range(0, NJE, 2):
                            lhsT = wq_sb[:, j:j + 2, mf * 128:(mf + 1) * 128]
                            rhs = h8[g][:, j:j + 2, :, :]
                            nc.tensor.matmul(pq[:], lhsT, rhs, perf_mode=DR,
                                             start=(j == 0), stop=(j == NJE - 2))
                        # split the two heads of this M-tile into padded slots
                        h0, h1 = 2 * mf, 2 * mf + 1
                        sl = slice(g * 512, (g + 1) * 512)
                        nc.scalar.activation(out=qT[0:64, h0, sl], in_=pq[0:64, :],
                                             func=AF.Identity,
                                             bias=cq_sb[0:64, mf:mf + 1])
                        nc.scalar.activation(out=qT[64:128, h1, sl], in_=pq[64:128, :],
                                             func=AF.Identity,
                                             bias=cq_sb[64:128, mf:mf + 1])
                for g in range(4):
                    for mf in range(NJE):
                        pk = qkps.tile([128, 512], f32, name="ps_qkv")
                        for j in range(0, NJE, 2):
                            lhsT = wk_sb[:, j:j + 2, mf * 128:(mf + 1) * 128]
                            rhs = h8[g][:, j:j + 2, :, :]
                            nc.tensor.matmul(pk[:], lhsT, rhs, perf_mode=DR,
                                             start=(j == 0), stop=(j == NJE - 2))
                        nc.scalar.activation(out=kT[:, mf, g * 512:(g + 1) * 512],
                                             in_=pk[:], func=AF.Identity,
                                             bias=ck_sb[:, mf:mf + 1])
                wv_sb = wqkv.tile([128, NJE, E], f8, name="wt")
                for j in range(NJE):
                    nc.gpsimd.dma_start(out=wv_sb[:, j, :], in_=wv_d[j])
                for st in range(NMT):
                    pv0 = qkps.tile([128, 512], f32, name="ps_qkv")
                    pv1 = qkps.tile([128, 512], f32, name="ps_qkv")
                    for j in range(0, NJE, 2):
                        lhsT = h8[st // 4][:, j:j + 2, st % 4, :]
                        nc.tensor.matmul(pv0[:], lhsT,
                                         wv_sb[:, j:j + 2, 0:512], perf_mode=DR,
                                         start=(j == 0), stop=(j == NJE - 2))
                        nc.tensor.matmul(pv1[:], lhsT,
                                         wv_sb[:, j:j + 2, 512:1024], perf_mode=DR,
                                         start=(j == 0), stop=(j == NJE - 2))
                    nc.vector.tensor_tensor(
                        out=v_aug[:, st, 0:8, 0:HS],
                        in0=pv0.rearrange("p (h d) -> p h d", h=8),
                        in1=cv_sb[:, 0:512].rearrange("p (h d) -> p h d", h=8),
                        op=OP.add)
                    nc.vector.tensor_tensor(
                        out=v_aug[:, st, 8:16, 0:HS],
                        in0=pv1.rearrange("p (h d) -> p h d", h=8),
                        in1=cv_sb[:, 512:1024].rearrange("p (h d) -> p h d", h=8),
                        op=OP.add)
                nc.gpsimd.memset(v_aug[:, :, :, HS:HS + 1], 1.0)
                for h in range(H):
                    p0 = 64 - (h % 2) * 64  # zero the OTHER head's rows
                    nc.gpsimd.memset(qT[p0:p0 + 64, h, :], 0.0)

        hT_es.close()

        # ---------- Phase 3: attention ----------
        oT_es = ExitStack()
        oT = oT_es.enter_context(tc.tile_pool(name="oT", bufs=1, side="left")) \
            .tile([128, NJE, TQ], f8)                  # normalized attn out^T
        # pre-issue the proj-phase loads on the sync queue BEFORE the r1
        # stores of the attention stream (engine DMA issue is in-order; a
        # store that waits on the attention pipeline would head-of-line
        # block these otherwise)
        wop_es = ExitStack()
        wop = wop_es.enter_context(tc.tile_pool(name="proj_w", bufs=1, side="left"))
        wo_sb = wop.tile([128, NJE, E], f8)
        for j in range(NJE):
            nc.sync.dma_start(out=wo_sb[:, j, :], in_=wo_d[j])
        h2T_es = ExitStack()
        h2T = h2T_es.enter_context(tc.tile_pool(name="h2T", bufs=1, side="left")) \
            .tile([128, NMQ, NJE, 128], bf16)
        pxp_es = ExitStack()
        pxp = pxp_es.enter_context(tc.tile_pool(name="proj_x", bufs=8, side="left"))
        x_tiles = []
        for mt in range(NMQ):
            x_sb = pxp.tile([128, E], f32, name="xq")
            nc.sync.dma_start(out=x_sb[:], in_=xq_d[mt * 128:(mt + 1) * 128, :])
            x_tiles.append(x_sb)
        with tc.tile_pool(name="att_ps", bufs=2, space="PSUM") as aps, \
             tc.tile_pool(name="att_po", bufs=2, space="PSUM") as ops, \
             tc.tile_pool(name="att_t", bufs=4, side="right") as atp, \
             tc.tile_pool(name="att_r", bufs=2, side="right") as rp, \
             tc.tile_pool(name="att_rb", bufs=2, side="right") as rbp:

            steps = [(h, st) for h in range(H) for st in range(NMT)]
            po_by_head = {}
            att_by_step = {}

            def emit_scores(h, st):
                ps = aps.tile([128, TQ], f32, name="ps_sc")
                lhsT = kT[:, h // 2, st * 128:(st + 1) * 128]
                nc.tensor.matmul(ps[:, 0:512], lhsT, qT[:, h, 0:512],
                                 start=True, stop=True)
                nc.tensor.matmul(ps[:, 512:1024], lhsT, qT[:, h, 512:1024],
                                 start=True, stop=True)
                at = atp.tile([128, TQ], bf16, name="att")
                if st in DVE_EXP_ST:
                    # approximate exp on DVE (bit-trick, ~3% rel err --
                    # softmax-normalized and averaged over 2048 keys the
                    # output impact is <0.1%); offloading these keeps the
                    # attention phase PE-bound instead of ACT-bound
                    t32 = atp.tile([128, TQ], i32, name="atti")
                    nc.vector.tensor_scalar(out=t32[:], in0=ps[:],
                                            scalar1=EXP_C1, scalar2=EXP_C2,
                                            op0=OP.mult, op1=OP.add)
                    nc.vector.tensor_copy(out=at[:], in_=t32.bitcast(f32)[:])
                else:
                    nc.scalar.activation(out=at[:], in_=ps[:], func=AF.Exp,
                                         scale=float(HS) ** -0.5)
                att_by_step[(h, st)] = at

            def emit_av(h, st):
                if st == 0:
                    po_by_head[h] = ops.tile([HS + 1, TQ], f32, name="ps_o")
                po = po_by_head[h]
                at = att_by_step.pop((h, st))
                vk = v_aug[:, st, h, :]
                nc.tensor.matmul(po[:, 0:512], vk, at[:, 0:512],
                                 start=(st == 0), stop=(st == NMT - 1))
                nc.tensor.matmul(po[:, 512:1024], vk, at[:, 512:1024],
                                 start=(st == 0), stop=(st == NMT - 1))
                if st == NMT - 1:
                    emit_head_finish(h, po)

            def emit_head_finish(h, po):
                r1 = rp.tile([1, TQ], f32, name="rsum")
                nc.vector.reciprocal(out=r1[:], in_=po[HS:HS + 1, :])
                nc.sync.dma_start(out=rbounce[h:h + 1, :], in_=r1[:])
                rb = rbp.tile([64, TQ], f32, name="rbc")
                nc.sync.dma_start(out=rb[:],
                                   in_=rbounce[h:h + 1, :].to_broadcast([64, TQ]))
                p0 = (h % 2) * 64
                nc.vector.tensor_tensor(out=oT[p0:p0 + 64, h // 2, :],
                                        in0=po[0:HS, :], in1=rb[:], op=OP.mult)

            for i, (h, st) in enumerate(steps):
                emit_scores(h, st)
                if i > 0:
                    emit_av(*steps[i - 1])
            emit_av(*steps[-1])

        qkv_es.close()

        # ---------- Phase 4+5: attn projection + residual, fused with LN2 ----------
        xr_pool = top.enter_context(tc.tile_pool(name="xr", bufs=1, side="right"))
        xr_t = [xr_pool.tile([128, E], f32, name=f"xr{i}") for i in range(NMQ)]
        with tc.tile_pool(name="proj_ps", bufs=4, space="PSUM") as pps, \
             tc.tile_pool(name="ln2s", bufs=10, side="left") as stp2, \
             tc.tile_pool(name="ln2h", bufs=5, side="left") as hbp2:
            for mt in range(NMQ):
                x_sb = x_tiles[mt]
                pa = pps.tile([128, 512], f32, name="ps_pr")
                pb = pps.tile([128, 512], f32, name="ps_pr")
                for j in range(0, NJE, 2):
                    lhsT = oT[:, j:j + 2, mt * 128:(mt + 1) * 128]
                    nc.tensor.matmul(pa[:], lhsT,
                                     wo_sb[:, j:j + 2, 0:512], perf_mode=DR,
                                     start=(j == 0), stop=(j == NJE - 2))
                    nc.tensor.matmul(pb[:], lhsT,
                                     wo_sb[:, j:j + 2, 512:1024], perf_mode=DR,
                                     start=(j == 0), stop=(j == NJE - 2))
                nc.vector.tensor_tensor(out=xr_t[mt][:, 0:512], in0=pa[:],
                                        in1=x_sb[:, 0:512], op=OP.add)
                nc.vector.tensor_tensor(out=xr_t[mt][:, 512:1024], in0=pb[:],
                                        in1=x_sb[:, 512:1024], op=OP.add)
                h_bf = hbp2.tile([128, E], bf16)
                layernorm(stp2, xr_t[mt][:], h_bf, eps_sb)
                nc.sync.dma_start_transpose(out=h2T[:, mt, :, :], in_=h_bf[:])
        pxp_es.close()

        # ---------- Phase 6: FFN1 (+bias, relu) ----------
        ffnT = top.enter_context(tc.tile_pool(name="ffnT", bufs=1, side="right")) \
            .tile([128, NJF, TQ], bf16)
        with tc.tile_pool(name="f1w", bufs=4, side="left") as f1wp, \
             tc.tile_pool(name="f1c", bufs=1, side="left") as f1cp, \
             tc.tile_pool(name="f1ps", bufs=3, space="PSUM") as f1ps:
            b1_sb = f1cp.tile([128, NJF], f32)
            nc.gpsimd.dma_start(out=b1_sb[:], in_=b1c_d[:])
            for mf in range(NJF):
                w1_sb = f1wp.tile([128, NJE, 128], bf16, name="w1t")
                nc.gpsimd.dma_start(out=w1_sb[:],
                                    in_=w1_d[mf].rearrange("p (j c) -> p j c", j=NJE))
                pf = f1ps.tile([128, TQ], f32, name="ps_f1")
                for j in range(NJE):
                    lhsT = w1_sb[:, j, :]
                    nc.tensor.matmul(pf[:, 0:512], lhsT, h2T[:, 0:4, j, :],
                                     start=(j == 0), stop=(j == NJE - 1))
                    nc.tensor.matmul(pf[:, 512:1024], lhsT, h2T[:, 4:8, j, :],
                                     start=(j == 0), stop=(j == NJE - 1))
                nc.scalar.activation(out=ffnT[:, mf, :], in_=pf[:], func=AF.Relu,
                                     bias=b1_sb[:, mf:mf + 1])

        h2T_es.close()
        wop_es.close()
        oT_es.close()

        # ---------- Phase 7: FFN2 + residual + b2 ----------
        with tc.tile_pool(name="f2w", bufs=8, side="left") as f2wp, \
             tc.tile_pool(name="f2c", bufs=1, side="left") as f2cp, \
             tc.tile_pool(name="f2o", bufs=3, side="left") as f2op, \
             tc.tile_pool(name="f2ps", bufs=8, space="PSUM") as f2ps:
            b2_sb = f2cp.tile([128, E], f32)
            nc.gpsimd.dma_start(out=b2_sb[:], in_=b2b_d[:])
            for nbh in range(2):
                psums = [f2ps.tile([128, 512], f32, name="ps_f2")
                         for _ in range(NMQ)]
                for k in range(NJF):
                    w2_sb = f2wp.tile([128, 512], bf16, name="w2t")
                    nc.gpsimd.dma_start(out=w2_sb[:],
                                        in_=w2_d[k][:, nbh * 512:(nbh + 1) * 512])
                    for mt in range(NMQ):
                        nc.tensor.matmul(psums[mt][:],
                                         ffnT[:, k, mt * 128:(mt + 1) * 128],
                                         w2_sb[:],
                                         start=(k == 0), stop=(k == NJF - 1))
                for mt in range(NMQ):
                    o_sb = f2op.tile([128, 512], f32, name="osb")
                    nc.vector.tensor_tensor(out=o_sb[:], in0=psums[mt][:],
                                            in1=xr_t[mt][:, nbh * 512:(nbh + 1) * 512],
                                            op=OP.add)
                    nc.vector.tensor_tensor(out=o_sb[:], in0=o_sb[:],
                                            in1=b2_sb[:, nbh * 512:(nbh + 1) * 512],
                                            op=OP.add)
                    nc.sync.dma_start(
                        out=out_d[mt * 128:(mt + 1) * 128, nbh * 512:(nbh + 1) * 512],
                        in_=o_sb[:])

        top.close()

    nc.compile()
    return nc


def _prep_weights(ln1_g, ln1_b, Wq, Wk, Wv, Wo, bo, ln2_g, ln2_b, W1, b1, W2, b2):
    f64 = np.float64
    g1 = np.asarray(ln1_g, f64)
    b1ln = np.asarray(ln1_b, f64)
    g2 = np.asarray(ln2_g, f64)
    b2ln = np.asarray(ln2_b, f64)

    def flat_qkv(W):
        return np.asarray(W, f64).transpose(1, 0, 2).reshape(E, H * HS)

    Wqf, Wkf, Wvf = flat_qkv(Wq), flat_qkv(Wk), flat_qkv(Wv)
    out = {}
    out["wq"] = np.ascontiguousarray((g1[:, None] * Wqf).reshape(NJE, 128, E).astype(F8))
    out["wk"] = np.ascontiguousarray((g1[:, None] * Wkf).reshape(NJE, 128, E).astype(F8))
    out["wv"] = np.ascontiguousarray((g1[:, None] * Wvf).reshape(NJE, 128, E).astype(F8))
    cq = (b1ln @ Wqf).astype(np.float32)
    ck = (b1ln @ Wkf).astype(np.float32)
    cv = (b1ln @ Wvf).astype(np.float32)
    out["cq"] = np.ascontiguousarray(cq.reshape(NJE, 128).T)
    out["ck"] = np.ascontiguousarray(ck.reshape(NJE, 128).T)
    out["cvb"] = np.ascontiguousarray(np.broadcast_to(cv, (128, E)))
    out["wo"] = np.ascontiguousarray(np.asarray(Wo, f64).reshape(NJE, 128, E).astype(F8))
    W1p = g2[:, None] * np.asarray(W1, f64)
    b1p = (np.asarray(b1, f64) + b2ln @ np.asarray(W1, f64)).astype(np.float32)
    out["w1"] = np.ascontiguousarray(
        W1p.reshape(NJE, 128, NJF, 128).transpose(2, 1, 0, 3).reshape(NJF, 128, E).astype(BF))
    out["b1c"] = np.ascontiguousarray(b1p.reshape(NJF, 128).T)
    out["w2"] = np.ascontiguousarray(np.asarray(W2, f64).reshape(NJF, 128, E).astype(BF))
    out["b2b"] = np.ascontiguousarray(
        np.broadcast_to(np.asarray(b2, np.float32), (128, E)))
    return out


def kernel(x, ln1_g, ln1_b, Wq, Wk, Wv, Wo, bo, ln2_g, ln2_b, W1, b1, W2, b2):
    global LAST_RESULTS
    from concourse.bass_utils import run_bass_kernel_spmd

    if "nc" not in _CACHE:
        _CACHE["nc"] = _build()
    nc = _CACHE["nc"]

    wmap = _prep_weights(ln1_g, ln1_b, Wq, Wk, Wv, Wo, bo,
                         ln2_g, ln2_b, W1, b1, W2, b2)
    x = np.asarray(x, np.float32)

    in_maps = []
    for c in range(NCORES):
        b, half = c // 2, c % 2
        xb = x[b]
        x_roll = np.ascontiguousarray(
            np.concatenate([xb[half * TQ:], xb[:half * TQ]], axis=0))
        m = dict(wmap)
        m["x"] = x_roll.astype(BF)
        m["xq"] = np.ascontiguousarray(
            x_roll[:TQ] + np.asarray(bo, np.float32)[None, :])
        in_maps.append(m)

    res = run_bass_kernel_spmd(nc, in_maps, list(range(NCORES)), trace=TRACE)
    LAST_RESULTS = res

    out = np.empty((B, T, E), np.float32)
    for c in range(NCORES):
        b, half = c // 2, c % 2
        out[b, half * TQ:(half + 1) * TQ] = res.results[c]["out"]
    return out


# revision 23
# speedup vs baseline: 1.1494x; 1.1494x over previous
"""Trainium2 Bass kernel for a pre-LN transformer block (B=4, T=2048, E=1024, H=16).

Sharding: 8 cores = 4 batches x 2 token-halves. Each core receives its batch's
full 2048 tokens (rolled so its own 1024 query tokens come first), computes
K/V for all 2048 tokens (redundantly with its pair core) and everything else
(Q, attention, proj, FFN) only for its own 1024 tokens. Zero cross-core
communication; host reassembles the output.

LayerNorm gains/biases are folded into the matmul weights host-side:
  q = LN1n(x) @ (diag(g1) Wq) + (b1_ln @ Wq)   with LN1n(x) = (x-mu)*rstd
so the device only computes (x-mu)*rstd. All matmuls run in bf16 (fp32
accumulation in PSUM); residuals/softmax stay fp32.
"""

import numpy as np
import ml_dtypes

BF = ml_dtypes.bfloat16
F8 = ml_dtypes.float8_e4m3

B, T, E, H, HS, FF = 4, 2048, 1024, 16, 64, 4096
TQ = T // 2          # own query tokens per core
NCORES = 8
EPS = 1e-5
NMT = T // 128       # 16 token tiles (full batch)
NMQ = TQ // 128      # 8 token tiles (own half)
NJE = E // 128       # 8 feature tiles of E
NJF = FF // 128      # 32 feature tiles of FF

_CACHE = {}
TRACE = False        # set by test harness to capture an NTFF profile
LAST_RESULTS = None  # BassKernelResults from the most recent run


def _build():
    import concourse.bacc as bacc
    import concourse.tile as tile
    from concourse import mybir
    from contextlib import ExitStack

    f32 = mybir.dt.float32
    bf16 = mybir.dt.bfloat16
    f8 = mybir.dt.float8e4
    DR = mybir.MatmulPerfMode.DoubleRow
    AF = mybir.ActivationFunctionType
    OP = mybir.AluOpType

    nc = bacc.Bacc("TRN2", target_bir_lowering=False, debug=False,
                   num_devices=NCORES)

    # ---- DRAM I/O ----
    x_d = nc.declare_dram_parameter("x", [T, E], bf16, isOutput=False)
    wq_d = nc.declare_dram_parameter("wq", [NJE, 128, E], f8, isOutput=False)
    wk_d = nc.declare_dram_parameter("wk", [NJE, 128, E], f8, isOutput=False)
    wv_d = nc.declare_dram_parameter("wv", [NJE, 128, E], f8, isOutput=False)
    wo_d = nc.declare_dram_parameter("wo", [NJE, 128, E], f8, isOutput=False)
    w1_d = nc.declare_dram_parameter("w1", [NJF, 128, E], bf16, isOutput=False)
    w2_d = nc.declare_dram_parameter("w2", [NJF, 128, E], bf16, isOutput=False)
    cq_d = nc.declare_dram_parameter("cq", [128, NJE], f32, isOutput=False)
    ck_d = nc.declare_dram_parameter("ck", [128, NJE], f32, isOutput=False)
    cvb_d = nc.declare_dram_parameter("cvb", [128, E], f32, isOutput=False)
    xq_d = nc.declare_dram_parameter("xq", [TQ, E], f32, isOutput=False)
    b2b_d = nc.declare_dram_parameter("b2b", [128, E], f32, isOutput=False)
    b1c_d = nc.declare_dram_parameter("b1c", [128, NJF], f32, isOutput=False)
    out_d = nc.declare_dram_parameter("out", [TQ, E], f32, isOutput=True)
    rbounce = nc.dram_tensor("rbounce", [H, TQ], f32)

    def layernorm(stats_pool, x_sb, out_bf, eps_sb):
        st = stats_pool.tile([128, 2, 6], f32, name="ln_st")
        nc.vector.bn_stats(out=st[:, 0, :], in_=x_sb[:, 0:512])
        nc.vector.bn_stats(out=st[:, 1, :], in_=x_sb[:, 512:1024])
        mv = stats_pool.tile([128, 2], f32, name="ln_mv")
        nc.vector.bn_aggr(out=mv[:], in_=st[:])
        rstd = stats_pool.tile([128, 1], f32, name="ln_rstd")
        nc.scalar.activation(out=rstd[:], in_=mv[:, 1:2], func=AF.Sqrt,
                             bias=eps_sb[:])
        nc.vector.reciprocal(out=rstd[:], in_=rstd[:])
        # out = x*rstd - mu*rstd on ACT (keeps the big elementwise op off DVE)
        nmr = stats_pool.tile([128, 1], f32, name="ln_nmr")
        nc.vector.tensor_tensor(out=nmr[:], in0=mv[:, 0:1], in1=rstd[:],
                                op=OP.mult)
        nc.vector.tensor_scalar_mul(out=nmr[:], in0=nmr[:], scalar1=-1.0)
        nc.scalar.activation(out=out_bf[:], in_=x_sb[:], func=AF.Identity,
                             bias=nmr[:], scale=rstd[:])

    with tile.TileContext(nc) as tc:
        top = ExitStack()
        const = top.enter_context(tc.tile_pool(name="const", bufs=1, side="left"))
        eps_sb = const.tile([128, 1], f32)
        nc.vector.memset(eps_sb[:], EPS)
        cq_sb = const.tile([128, NJE], f32)
        nc.sync.dma_start(out=cq_sb[:], in_=cq_d[:])
        ck_sb = const.tile([128, NJE], f32)
        nc.sync.dma_start(out=ck_sb[:], in_=ck_d[:])
        cv_sb = const.tile([128, E], f32)
        nc.sync.dma_start(out=cv_sb[:], in_=cvb_d[:])

        # Persistent activations use per-tensor pools whose lifetimes are
        # managed manually (ExitStack.close as soon as the last consumer
        # phase is emitted) so SBUF space is recycled across phases. Pool
        # allocation is a two-sided LIFO stack; sides are chosen so pool
        # lifetimes nest properly.

        # ---------- Phase 1: LN1 + transpose ----------
        hT_es = ExitStack()
        hT_pool = hT_es.enter_context(tc.tile_pool(name="hT", bufs=1, side="left"))
        # 4 tiles of 4 token-tiles each so QKV matmuls can start as soon as
        # the first group of LN1 transposes lands
        hG = [hT_pool.tile([128, 4, NJE, 128], bf16, name=f"hG{g}")
              for g in range(4)]
        h8 = [hT_pool.tile([128, NJE, 4, 128], f8, name=f"h8{g}")
              for g in range(4)]
        with tc.tile_pool(name="ln1", bufs=16, side="left") as xin, \
             tc.tile_pool(name="ln1s", bufs=10, side="left") as stp, \
             tc.tile_pool(name="ln1h", bufs=5, side="left") as hbp:
            xt = []
            for mt in range(NMT):
                x_sb = xin.tile([128, E], bf16)
                nc.sync.dma_start(out=x_sb[:], in_=x_d[mt * 128:(mt + 1) * 128, :])
                xt.append(x_sb)
            for mt in range(NMT):
                h_bf = hbp.tile([128, E], bf16)
                layernorm(stp, xt[mt], h_bf, eps_sb)
                nc.sync.dma_start_transpose(out=hG[mt // 4][:, mt % 4, :, :],
                                            in_=h_bf[:])
                nc.vector.tensor_copy(out=h8[mt // 4][:, :, mt % 4, :],
                                      in_=hG[mt // 4][:, mt % 4, :, :])

        # ---------- Phase 2: QKV projections ----------
        qkv_es = ExitStack()
        qkv_pool = qkv_es.enter_context(tc.tile_pool(name="qkvact", bufs=1, side="right"))
        # q^T is stored zero-padded per head: head h occupies partition rows
        # (h%2)*64..+64 of slice [:, h, :], the other 64 rows are zero. The
        # scores matmul can then contract over the full K=128 partitions
        # (the paired head's k rows hit zeros) which keeps the PE HAM
        # activity monitor at full clock; K=64 matmuls run the array at
        # half activity and HAM throttles the PE to 1.2 GHz.
        qT = qkv_pool.tile([128, H, TQ], bf16)         # q^T (own tokens)
        kT = qkv_pool.tile([128, NJE, T], bf16)        # k^T (all tokens)
        v_aug = qkv_pool.tile([128, NMT, H, HS + 1], bf16)  # v + ones col
        with tc.tile_pool(name="qkv_ps", bufs=8, space="PSUM") as qkps, \
             tc.tile_pool(name="w_pool", bufs=2, side="right") as wqkv:
                wq_sb = wqkv.tile([128, NJE, E], f8, name="wt")
                wk_sb = wqkv.tile([128, NJE, E], f8, name="wt")
                for j in range(NJE):
                    nc.gpsimd.dma_start(out=wq_sb[:, j, :], in_=wq_d[j])
                for j in range(NJE):
                    nc.gpsimd.dma_start(out=wk_sb[:, j, :], in_=wk_d[j])
                for g in range(2):
                    for mf in range(NJE):
                        pq = qkps.tile([128, 512], f32, name="ps_qkv")
                        for j in range(0, NJE, 2):
                            lhsT = wq_sb[:, j:j + 2, mf * 128:(mf + 1) * 128]
                            rhs = h8[g][:, j:j + 2, :, :]
                            nc.tensor.matmul(pq[:], lhsT, rhs, perf_mode=DR,
                                             start=(j == 0), stop=(j == NJE - 2))
                        # split the two heads of this M-tile into padded slots
                        h0, h1 = 2 * mf, 2 * mf + 1
                        sl = slice(g * 512, (g + 1) * 512)
                        nc.scalar.activation(out=qT[0:64, h0, sl], in_=pq[0:64, :],
                                             func=AF.Identity,
                                             bias=cq_sb[0:64, mf:mf + 1])
                        nc.scalar.activation(out=qT[64:128, h1, sl], in_=pq[64:128, :],
                                             func=AF.Identity,
                                             bias=cq_sb[64:128, mf:mf + 1])
                for g in range(4):
                    for mf in range(NJE):
                        pk = qkps.tile([128, 512], f32, name="ps_qkv")
                        for j in range(0, NJE, 2):
                            lhsT = wk_sb[:, j:j + 2, mf * 128:(mf + 1) * 128]
                            rhs = h8[g][:, j:j + 2, :, :]
                            nc.tensor.matmul(pk[:], lhsT, rhs, perf_mode=DR,
                                             start=(j == 0), stop=(j == NJE - 2))
                        nc.scalar.activation(out=kT[:, mf, g * 512:(g + 1) * 512],
                                             in_=pk[:], func=AF.Identity,
                                             bias=ck_sb[:, mf:mf + 1])
                wv_sb = wqkv.tile([128, NJE, E], f8, name="wt")
                for j in range(NJE):
                    nc.gpsimd.dma_start(out=wv_sb[:, j, :], in_=wv_d[j])
                for st in range(NMT):
                    pv0 = qkps.tile([128, 512], f32, name="ps_qkv")
                    pv1 = qkps.tile([128, 512], f32, name="ps_qkv")
                    for j in range(0, NJE, 2):
                        lhsT = h8[st // 4][:, j:j + 2, st % 4, :]
                        nc.tensor.matmul(pv0[:], lhsT,
                                         wv_sb[:, j:j + 2, 0:512], perf_mode=DR,
                                         start=(j == 0), stop=(j == NJE - 2))
                        nc.tensor.matmul(pv1[:], lhsT,
                                         wv_sb[:, j:j + 2, 512:1024], perf_mode=DR,
                                         start=(j == 0), stop=(j == NJE - 2))
                    nc.vector.tensor_tensor(
                        out=v_aug[:, st, 0:8, 0:HS],
                        in0=pv0.rearrange("p (h d) -> p h d", h=8),
                        in1=cv_sb[:, 0:512].rearrange("p (h d) -> p h d", h=8),
                        op=OP.add)
                    nc.vector.tensor_tensor(
                        out=v_aug[:, st, 8:16, 0:HS],
                        in0=pv1.rearrange("p (h d) -> p h d", h=8),
                        in1=cv_sb[:, 512:1024].rearrange("p (h d) -> p h d", h=8),
                        op=OP.add)
                nc.gpsimd.memset(v_aug[:, :, :, HS:HS + 1], 1.0)
                for h in range(H):
                    p0 = 64 - (h % 2) * 64  # zero the OTHER head's rows
                    nc.gpsimd.memset(qT[p0:p0 + 64, h, :], 0.0)

        hT_es.close()

        # ---------- Phase 3: attention ----------
        oT_es = ExitStack()
        oT = oT_es.enter_context(tc.tile_pool(name="oT", bufs=1, side="left")) \
            .tile([128, NJE, TQ], f8)                  # normalized attn out^T
        # pre-issue the proj-phase loads on the sync queue BEFORE the r1
        # stores of the attention stream (engine DMA issue is in-order; a
        # store that waits on the attention pipeline would head-of-line
        # block these otherwise)
        wop_es = ExitStack()
        wop = wop_es.enter_context(tc.tile_pool(name="proj_w", bufs=1, side="left"))
        wo_sb = wop.tile([128, NJE, E], f8)
        for j in range(NJE):
            nc.sync.dma_start(out=wo_sb[:, j, :], in_=wo_d[j])
        h2T_es = ExitStack()
        h2T = h2T_es.enter_context(tc.tile_pool(name="h2T", bufs=1, side="left")) \
            .tile([128, NMQ, NJE, 128], bf16)
        pxp_es = ExitStack()
        pxp = pxp_es.enter_context(tc.tile_pool(name="proj_x", bufs=8, side="left"))
        x_tiles = []
        for mt in range(NMQ):
            x_sb = pxp.tile([128, E], f32, name="xq")
            nc.sync.dma_start(out=x_sb[:], in_=xq_d[mt * 128:(mt + 1) * 128, :])
            x_tiles.append(x_sb)
        with tc.tile_pool(name="att_ps", bufs=2, space="PSUM") as aps, \
             tc.tile_pool(name="att_po", bufs=2, space="PSUM") as ops, \
             tc.tile_pool(name="att_t", bufs=4, side="right") as atp, \
             tc.tile_pool(name="att_r", bufs=2, side="right") as rp, \
             tc.tile_pool(name="att_rb", bufs=2, side="right") as rbp:

            steps = [(h, st) for h in range(H) for st in range(NMT)]
            po_by_head = {}
            att_by_step = {}

            def emit_scores(h, st):
                ps = aps.tile([128, TQ], f32, name="ps_sc")
                lhsT = kT[:, h // 2, st * 128:(st + 1) * 128]
                nc.tensor.matmul(ps[:, 0:512], lhsT, qT[:, h, 0:512],
                                 start=True, stop=True)
                nc.tensor.matmul(ps[:, 512:1024], lhsT, qT[:, h, 512:1024],
                                 start=True, stop=True)
                at = atp.tile([128, TQ], bf16, name="att")
                nc.scalar.activation(out=at[:], in_=ps[:], func=AF.Exp,
                                     scale=float(HS) ** -0.5)
                att_by_step[(h, st)] = at

            def emit_av(h, st):
                if st == 0:
                    po_by_head[h] = ops.tile([HS + 1, TQ], f32, name="ps_o")
                po = po_by_head[h]
                at = att_by_step.pop((h, st))
                vk = v_aug[:, st, h, :]
                nc.tensor.matmul(po[:, 0:512], vk, at[:, 0:512],
                                 start=(st == 0), stop=(st == NMT - 1))
                nc.tensor.matmul(po[:, 512:1024], vk, at[:, 512:1024],
                                 start=(st == 0), stop=(st == NMT - 1))
                if st == NMT - 1:
                    emit_head_finish(h, po)

            def emit_head_finish(h, po):
                r1 = rp.tile([1, TQ], f32, name="rsum")
                nc.vector.reciprocal(out=r1[:], in_=po[HS:HS + 1, :])
                nc.sync.dma_start(out=rbounce[h:h + 1, :], in_=r1[:])
                rb = rbp.tile([64, TQ], f32, name="rbc")
                nc.sync.dma_start(out=rb[:],
                                   in_=rbounce[h:h + 1, :].to_broadcast([64, TQ]))
                p0 = (h % 2) * 64
                nc.vector.tensor_tensor(out=oT[p0:p0 + 64, h // 2, :],
                                        in0=po[0:HS, :], in1=rb[:], op=OP.mult)

            for i, (h, st) in enumerate(steps):
                emit_scores(h, st)
                if i > 0:
                    emit_av(*steps[i - 1])
            emit_av(*steps[-1])

        qkv_es.close()

        # ---------- Phase 4+5: attn projection + residual, fused with LN2 ----------
        xr_pool = top.enter_context(tc.tile_pool(name="xr", bufs=1, side="right"))
        xr_t = [xr_pool.tile([128, E], f32, name=f"xr{i}") for i in range(NMQ)]
        with tc.tile_pool(name="proj_ps", bufs=4, space="PSUM") as pps, \
             tc.tile_pool(name="ln2s", bufs=10, side="left") as stp2, \
             tc.tile_pool(name="ln2h", bufs=5, side="left") as hbp2:
            for mt in range(NMQ):
                x_sb = x_tiles[mt]
                pa = pps.tile([128, 512], f32, name="ps_pr")
                pb = pps.tile([128, 512], f32, name="ps_pr")
                for j in range(0, NJE, 2):
                    lhsT = oT[:, j:j + 2, mt * 128:(mt + 1) * 128]
                    nc.tensor.matmul(pa[:], lhsT,
                                     wo_sb[:, j:j + 2, 0:512], perf_mode=DR,
                                     start=(j == 0), stop=(j == NJE - 2))
                    nc.tensor.matmul(pb[:], lhsT,
                                     wo_sb[:, j:j + 2, 512:1024], perf_mode=DR,
                                     start=(j == 0), stop=(j == NJE - 2))
                nc.vector.tensor_tensor(out=xr_t[mt][:, 0:512], in0=pa[:],
                                        in1=x_sb[:, 0:512], op=OP.add)
                nc.vector.tensor_tensor(out=xr_t[mt][:, 512:1024], in0=pb[:],
                                        in1=x_sb[:, 512:1024], op=OP.add)
                h_bf = hbp2.tile([128, E], bf16)
                layernorm(stp2, xr_t[mt][:], h_bf, eps_sb)
                nc.sync.dma_start_transpose(out=h2T[:, mt, :, :], in_=h_bf[:])
        pxp_es.close()

        # ---------- Phase 6: FFN1 (+bias, relu) ----------
        ffnT = top.enter_context(tc.tile_pool(name="ffnT", bufs=1, side="right")) \
            .tile([128, NJF, TQ], bf16)
        with tc.tile_pool(name="f1w", bufs=4, side="left") as f1wp, \
             tc.tile_pool(name="f1c", bufs=1, side="left") as f1cp, \
             tc.tile_pool(name="f1ps", bufs=3, space="PSUM") as f1ps:
            b1_sb = f1cp.tile([128, NJF], f32)
            nc.gpsimd.dma_start(out=b1_sb[:], in_=b1c_d[:])
            for mf in range(NJF):
                w1_sb = f1wp.tile([128, NJE, 128], bf16, name="w1t")
                nc.gpsimd.dma_start(out=w1_sb[:],
                                    in_=w1_d[mf].rearrange("p (j c) -> p j c", j=NJE))
                pf = f1ps.tile([128, TQ], f32, name="ps_f1")
                for j in range(NJE):
                    lhsT = w1_sb[:, j, :]
                    nc.tensor.matmul(pf[:, 0:512], lhsT, h2T[:, 0:4, j, :],
                                     start=(j == 0), stop=(j == NJE - 1))
                    nc.tensor.matmul(pf[:, 512:1024], lhsT, h2T[:, 4:8, j, :],
                                     start=(j == 0), stop=(j == NJE - 1))
                nc.scalar.activation(out=ffnT[:, mf, :], in_=pf[:], func=AF.Relu,
                                     bias=b1_sb[:, mf:mf + 1])

        h2T_es.close()
        wop_es.close()
        oT_es.close()

        # ---------- Phase 7: FFN2 + residual + b2 ----------
        with tc.tile_pool(name="f2w", bufs=8, side="left") as f2wp, \
             tc.tile_pool(name="f2c", bufs=1, side="left") as f2cp, \
             tc.tile_pool(name="f2o", bufs=3, side="left") as f2op, \
             tc.tile_pool(name="f2ps", bufs=8, space="PSUM") as f2ps:
            b2_sb = f2cp.tile([128, E], f32)
            nc.gpsimd.dma_start(out=b2_sb[:], in_=b2b_d[:])
            for nbh in range(2):
                psums = [f2ps.tile([128, 512], f32, name="ps_f2")
                         for _ in range(NMQ)]
                for k in range(NJF):
                    w2_sb = f2wp.tile([128, 512], bf16, name="w2t")
                    nc.gpsimd.dma_start(out=w2_sb[:],
                                        in_=w2_d[k][:, nbh * 512:(nbh + 1) * 512])
                    for mt in range(NMQ):
                        nc.tensor.matmul(psums[mt][:],
                                         ffnT[:, k, mt * 128:(mt + 1) * 128],
                                         w2_sb[:],
                                         start=(k == 0), stop=(k == NJF - 1))
                for mt in range(NMQ):
                    o_sb = f2op.tile([128, 512], f32, name="osb")
                    nc.vector.tensor_tensor(out=o_sb[:], in0=psums[mt][:],
                                            in1=xr_t[mt][:, nbh * 512:(nbh + 1) * 512],
                                            op=OP.add)
                    nc.vector.tensor_tensor(out=o_sb[:], in0=o_sb[:],
                                            in1=b2_sb[:, nbh * 512:(nbh + 1) * 512],
                                            op=OP.add)
                    nc.sync.dma_start(
                        out=out_d[mt * 128:(mt + 1) * 128, nbh * 512:(nbh + 1) * 512],
                        in_=o_sb[:])

        top.close()

    nc.compile()
    return nc


def _prep_weights(ln1_g, ln1_b, Wq, Wk, Wv, Wo, bo, ln2_g, ln2_b, W1, b1, W2, b2):
    f64 = np.float64
    g1 = np.asarray(ln1_g, f64)
    b1ln = np.asarray(ln1_b, f64)
    g2 = np.asarray(ln2_g, f64)
    b2ln = np.asarray(ln2_b, f64)

    def flat_qkv(W):
        return np.asarray(W, f64).transpose(1, 0, 2).reshape(E, H * HS)

    Wqf, Wkf, Wvf = flat_qkv(Wq), flat_qkv(Wk), flat_qkv(Wv)
    out = {}
    out["wq"] = np.ascontiguousarray((g1[:, None] * Wqf).reshape(NJE, 128, E).astype(F8))
    out["wk"] = np.ascontiguousarray((g1[:, None] * Wkf).reshape(NJE, 128, E).astype(F8))
    out["wv"] = np.ascontiguousarray((g1[:, None] * Wvf).reshape(NJE, 128, E).astype(F8))
    cq = (b1ln @ Wqf).astype(np.float32)
    ck = (b1ln @ Wkf).astype(np.float32)
    cv = (b1ln @ Wvf).astype(np.float32)
    out["cq"] = np.ascontiguousarray(cq.reshape(NJE, 128).T)
    out["ck"] = np.ascontiguousarray(ck.reshape(NJE, 128).T)
    out["cvb"] = np.ascontiguousarray(np.broadcast_to(cv, (128, E)))
    out["wo"] = np.ascontiguousarray(np.asarray(Wo, f64).reshape(NJE, 128, E).astype(F8))
    W1p = g2[:, None] * np.asarray(W1, f64)
    b1p = (np.asarray(b1, f64) + b2ln @ np.asarray(W1, f64)).astype(np.float32)
    out["w1"] = np.ascontiguousarray(
        W1p.reshape(NJE, 128, NJF, 128).transpose(2, 1, 0, 3).reshape(NJF, 128, E).astype(BF))
    out["b1c"] = np.ascontiguousarray(b1p.reshape(NJF, 128).T)
    out["w2"] = np.ascontiguousarray(np.asarray(W2, f64).reshape(NJF, 128, E).astype(BF))
    out["b2b"] = np.ascontiguousarray(
        np.broadcast_to(np.asarray(b2, np.float32), (128, E)))
    return out


def kernel(x, ln1_g, ln1_b, Wq, Wk, Wv, Wo, bo, ln2_g, ln2_b, W1, b1, W2, b2):
    global LAST_RESULTS
    from concourse.bass_utils import run_bass_kernel_spmd

    if "nc" not in _CACHE:
        _CACHE["nc"] = _build()
    nc = _CACHE["nc"]

    wmap = _prep_weights(ln1_g, ln1_b, Wq, Wk, Wv, Wo, bo,
                         ln2_g, ln2_b, W1, b1, W2, b2)
    x = np.asarray(x, np.float32)

    in_maps = []
    for c in range(NCORES):
        b, half = c // 2, c % 2
        xb = x[b]
        x_roll = np.ascontiguousarray(
            np.concatenate([xb[half * TQ:], xb[:half * TQ]], axis=0))
        m = dict(wmap)
        m["x"] = x_roll.astype(BF)
        m["xq"] = np.ascontiguousarray(
            x_roll[:TQ] + np.asarray(bo, np.float32)[None, :])
        in_maps.append(m)

    res = run_bass_kernel_spmd(nc, in_maps, list(range(NCORES)), trace=TRACE)
    LAST_RESULTS = res

    out = np.empty((B, T, E), np.float32)
    for c in range(NCORES):
        b, half = c // 2, c % 2
        out[b, half * TQ:(half + 1) * TQ] = res.results[c]["out"]
    return out
